# revision 1
# baseline (speedup 1.0000x reference)
"""ColourLoss Trainium2 kernel (self-contained).

Computes, per batch sample b:
    loss[b] = emd(hist_g(img), hist_g(img_t)) + emd(hist_b(img), hist_b(img_t))
(emd(r_hist, r_hist) == 0 exactly, so the r channel is skipped.)

Math: the soft-histogram bin memberships telescope under cumsum:
    cumsum_k pj = sigmoid(2.5*t) - sigmoid(2.5*(t - (k+1))),  t = 255*x
so  cdf[k] = (T0 - F[k+1])/N  with  F[m] = sum_n sigmoid(2.5*(t_n - m)).

F is computed with a radix-16 decomposition: per-pixel block a = floor(t/16),
offset xi = t - 16a in [0,16).  22 sigmoid "slot" columns per pixel (scalar
engine) cover every non-saturated (pixel, bin) pair; a one-hot-over-blocks
matmul (tensor engine, contraction over pixels) bins them into per-block sums
G[16, 24]; saturated pairs are recovered exactly from block counts (suffix
sums).  Sharding: batch (8 samples) across 8 NeuronCores; each core handles
its sample's 4 channel-images.
"""
from contextlib import ExitStack

import numpy as np

import concourse.bass as bass
import concourse.tile as tile
from concourse import bacc, bass_utils, mybir

F32 = mybir.dt.float32
F16 = mybir.dt.float16
F8 = mybir.dt.float8e4
I32 = mybir.dt.int32

P = 128        # SBUF partitions
FD = 512       # free elems per partition per channel-image (128*512 = 65536)
NPIX = 65536
NCH = 4        # channel-images per core: [g, g_t, b, b_t]
N_CORES = 8

# Sigmoid slot columns: value sigmoid(2.5*(xi - off)).
#   cols 0..15:  off = b          (own block)
#   cols 16..18: off = 16 + b, b in {0,1,2}    (pixel one block below bin)
#   cols 19..21: off = b - 16, b in {13,14,15} (pixel one block above bin)
SLOT_OFFS = [float(b) for b in range(16)] + [16.0, 17.0, 18.0] + [-3.0, -2.0, -1.0]
COL_ONES = 22
NCOL = 23


def _colour_loss_kernel(
    ctx: ExitStack, tc: "tile.TileContext", out_ap, chs_ap, dbg_ap=None, reps=1
):
    nc = tc.nc
    AOP = mybir.AluOpType

    consts = ctx.enter_context(tc.tile_pool(name="consts", bufs=1))
    prep = ctx.enter_context(tc.tile_pool(name="prep", bufs=2))
    bigs = ctx.enter_context(tc.tile_pool(name="bigs", bufs=2))
    psums = ctx.enter_context(tc.tile_pool(name="psums", bufs=1, space="PSUM"))
    asm = ctx.enter_context(tc.tile_pool(name="asm", bufs=1))
    pools = (consts, prep, bigs, psums, asm)

    # constant iota row 0..15 (f32), same in every partition
    iota_i = consts.tile([P, 16], I32)
    nc.gpsimd.iota(iota_i[:], pattern=[[1, 16]], base=0, channel_multiplier=0)
    iota_f = consts.tile([P, 16], F32)
    nc.vector.tensor_copy(iota_f[:], iota_i[:])

    # per-slot activation biases (-2.5 * off), broadcast to all partitions
    nslot = len(SLOT_OFFS)
    bias_dram = nc.inline_tensor(
        np.array([[-2.5 * off for off in SLOT_OFFS]], dtype=np.float32),
        name="slotbias",
    ).ap()
    bias_sb = consts.tile([P, nslot], F32)
    nc.sync.dma_start(
        bias_sb[:],
        bass.AP(
            tensor=bias_dram.tensor,
            offset=bias_dram.offset,
            ap=[[0, P], bias_dram.ap[1]],
        ),
    )

    for _ in range(reps):
        _colour_loss_once(tc, out_ap, chs_ap, dbg_ap, pools, iota_f, bias_sb)


def _colour_loss_once(tc, out_ap, chs_ap, dbg_ap, pools, iota_f, bias_sb):
    nc = tc.nc
    AOP = mybir.AluOpType
    ACT = mybir.ActivationFunctionType
    consts, prep, bigs, psums, asm = pools
    FD2 = 2 * FD  # two channel-images per batch

    gpsums = [None] * NCH
    for q in range(2):  # pairs: (g, g_t), (b, b_t)
        xt = prep.tile([P, 2, FD], F32, tag="xt")
        nc.sync.dma_start(xt[:, 0, :], chs_ap[2 * q])
        nc.sync.dma_start(xt[:, 1, :], chs_ap[2 * q + 1])
        xtf = xt[:].rearrange("p h f -> p (h f)")
        t = prep.tile([P, FD2], F32, tag="t")
        nc.vector.tensor_scalar_mul(t[:], xtf, 255.0)
        tdiv = prep.tile([P, FD2], F32, tag="tdiv")
        nc.vector.tensor_scalar_mul(tdiv[:], t[:], 0.0625)
        a_i = prep.tile([P, FD2], I32, tag="a_i")
        nc.vector.tensor_copy(a_i[:], tdiv[:])  # trunc in sim, round-nearest on HW
        a_f0 = prep.tile([P, FD2], F32, tag="a_f0")
        nc.vector.tensor_copy(a_f0[:], a_i[:])
        xi0 = prep.tile([P, FD2], F32, tag="xi0")
        nc.vector.scalar_tensor_tensor(
            xi0[:], in0=a_f0[:], scalar=-16.0, in1=t[:],
            op0=AOP.mult, op1=AOP.add,
        )
        # fixup: convert may have given floor(t/16)+1 -> xi0 in [-16, 0).
        # neg = (xi0 < 0); a_f = a_f0 - neg; xi = xi0 + 16*neg
        neg = prep.tile([P, FD2], F32, tag="neg")
        nc.vector.tensor_scalar(neg[:], xi0[:], 0.0, None, op0=AOP.is_lt)
        a_f = prep.tile([P, FD2], F32, tag="a_f")
        nc.vector.tensor_tensor(a_f[:], a_f0[:], neg[:], op=AOP.subtract)
        xi = prep.tile([P, FD2], F32, tag="xi")
        nc.vector.scalar_tensor_tensor(
            xi[:], in0=neg[:], scalar=16.0, in1=xi0[:],
            op0=AOP.mult, op1=AOP.add,
        )

        # one-hot over blocks: ind[p, s, j] = (a[p, j] == s), fp8 (exact 0/1).
        # 16 tensor_scalar is_equal instrs, split DVE / GPSIMD.
        ind = bigs.tile([P, 16, FD2], F8, tag="ind")
        for s in range(16):
            eng = nc.vector if s % 2 == 0 else nc.gpsimd
            eng.tensor_scalar(
                ind[:, s, :], a_f[:], float(s), None, op0=AOP.is_equal
            )

        # slot values
        u = bigs.tile([P, NCOL, FD2], F16, tag="u")
        for s in range(len(SLOT_OFFS)):
            nc.scalar.activation(
                u[:, s, :], xi[:], ACT.Sigmoid, bias=bias_sb[:, s : s + 1], scale=2.5
            )
        nc.gpsimd.memset(u[:, COL_ONES, :], 1.0)

        # bin: G[a, col] = sum over pixels in block a of u[pixel, col]
        for h in range(2):
            i = 2 * q + h
            g = psums.tile([16, NCOL], F32, tag=f"g{i}")
            for j in range(FD):
                hj = h * FD + j
                nc.tensor.matmul(
                    g[:],
                    ind[:, :, hj : hj + 1],
                    u[:, :, hj : hj + 1],
                    start=(j == 0),
                    stop=(j == FD - 1),
                )
            gpsums[i] = g

    # ---- assembly ----
    gsb = []
    for i in range(NCH):
        s = asm.tile([16, NCOL], F32, tag=f"gsb{i}")
        nc.vector.tensor_copy(s[:], gpsums[i][:])
        if dbg_ap is not None:
            nc.sync.dma_start(dbg_ap[i], s[:])
        gsb.append(s)
    gd0 = asm.tile([16, NCOL], F32)
    nc.vector.tensor_tensor(gd0[:], gsb[0][:], gsb[1][:], op=AOP.subtract)
    gd1 = asm.tile([16, NCOL], F32)
    nc.vector.tensor_tensor(gd1[:], gsb[2][:], gsb[3][:], op=AOP.subtract)

    # flatten both pair-diffs onto partition 0: gdf[1, pair, A, col]
    gdf = asm.tile([1, 2, 16, NCOL], F32)
    nc.sync.dma_start(gdf[:, 0], gd0[:])
    nc.sync.dma_start(gdf[:, 1], gd1[:])

    # counts and suffix sums (zero-padded to 32)
    cnt0 = asm.tile([1, 2, 16], F32)
    nc.vector.tensor_copy(cnt0[:], gdf[:, :, :, COL_ONES])
    cntp = asm.tile([1, 2, 32], F32)
    nc.vector.memset(cntp[:], 0.0)
    nc.vector.tensor_copy(cntp[:, :, 0:16], cnt0[:])
    # in-place doubling: cntp[i] = sum_{j >= i} cnt[j]
    for k in (1, 2, 4, 8):
        nc.vector.tensor_tensor(
            cntp[:, :, 0 : 32 - k],
            cntp[:, :, 0 : 32 - k],
            cntp[:, :, k:32],
            op=AOP.add,
        )

    # D[m], m = 16A + b, laid out [1, pair, 256] with 4D view [1, pair, A, b]
    d = asm.tile([1, 2, 256], F32)
    d4 = d[:].rearrange("x p (A b) -> x p A b", b=16)
    nc.vector.tensor_copy(d4, gdf[:, :, :, 0:16])
    nc.vector.tensor_tensor(
        d4[:, :, 1:16, 0:3], d4[:, :, 1:16, 0:3], gdf[:, :, 0:15, 16:19], op=AOP.add
    )
    nc.vector.tensor_tensor(
        d4[:, :, 0:15, 13:16], d4[:, :, 0:15, 13:16], gdf[:, :, 1:16, 19:22], op=AOP.add
    )
    # + suffix count of blocks >= A+2
    nc.vector.tensor_tensor(
        d4, d4, cntp[:, :, 2:18].unsqueeze(3).broadcast_to([1, 2, 16, 16]), op=AOP.add
    )
    # + cnt[A+1] for b <= 12
    nc.vector.tensor_tensor(
        d4[:, :, 0:15, 0:13],
        d4[:, :, 0:15, 0:13],
        cnt0[:, :, 1:16].unsqueeze(3).broadcast_to([1, 2, 15, 13]),
        op=AOP.add,
    )

    # T0 diff per pair: T0 = (N - cnt[0]) + G[0, col0]  (sigmoid(2.5t) == 1.0
    # exactly in f32 for t >= 16, i.e. every pixel outside block 0)
    t0d = asm.tile([1, 2, 1], F32)
    nc.vector.tensor_tensor(
        t0d[:], gdf[:, :, 0, 0:1], cnt0[:, :, 0:1], op=AOP.subtract
    )
    invn = 1.0 / float(NPIX)
    t0dn = asm.tile([1, 2, 1], F32)
    nc.vector.tensor_scalar_mul(t0dn[:], t0d[:], invn)

    # delta[m] = (T0d - D[m])/N ; loss = sum over pairs, m=1..255 of delta^2
    tmp = asm.tile([1, 2, 256], F32)
    nc.vector.scalar_tensor_tensor(
        tmp[:], in0=d[:], scalar=-invn, in1=t0dn[:].broadcast_to([1, 2, 256]),
        op0=AOP.mult, op1=AOP.add,
    )
    dummy = asm.tile([1, 2, 255], F32)
    lossacc = asm.tile([1, 1], F32)
    nc.scalar.activation(
        dummy[:], tmp[:, :, 1:256], ACT.Square, accum_out=lossacc[:]
    )
    nc.sync.dma_start(out_ap[:], lossacc[:])


_CACHE: dict = {}


def build_nc(reps: int = 1):
    key = ("nc", reps)
    if key in _CACHE:
        return _CACHE[key]
    nc = bacc.Bacc(
        "TRN2", target_bir_lowering=False, debug=False, num_devices=N_CORES
    )
    chs = nc.dram_tensor("chs", [NCH, P, FD], F32, kind="ExternalInput").ap()
    out = nc.dram_tensor("out", [1, 1], F32, kind="ExternalOutput").ap()
    with tile.TileContext(nc) as tc:
        with ExitStack() as ctx:
            _colour_loss_kernel(ctx, tc, out, chs, reps=reps)
    nc.compile()
    _CACHE[key] = nc
    return nc


def make_in_maps(img: np.ndarray, img_t: np.ndarray):
    img = np.asarray(img)
    img_t = np.asarray(img_t)
    in_maps = []
    for c in range(N_CORES):
        chs = np.stack(
            [
                img[c, 1].reshape(P, FD),
                img_t[c, 1].reshape(P, FD),
                img[c, 2].reshape(P, FD),
                img_t[c, 2].reshape(P, FD),
            ],
            axis=0,
        ).astype(np.float32)
        in_maps.append({"chs": np.ascontiguousarray(chs)})
    return in_maps


def kernel(img: np.ndarray, img_t: np.ndarray, trace: bool = False):
    nc = build_nc()
    in_maps = make_in_maps(img, img_t)
    res = bass_utils.run_bass_kernel_spmd(
        nc, in_maps, core_ids=list(range(N_CORES)), trace=trace
    )
    out = np.array(
        [res.results[c]["out"][0, 0] for c in range(N_CORES)], dtype=np.float32
    )
    if trace:
        kernel.last_results = res  # type: ignore[attr-defined]
    return out



# revision 6
# speedup vs baseline: 1.8923x; 1.8923x over previous
"""ColourLoss Trainium2 kernel (self-contained).

Computes, per batch sample b:
    loss[b] = emd(hist_g(img), hist_g(img_t)) + emd(hist_b(img), hist_b(img_t))
(emd(r_hist, r_hist) == 0 exactly, so the r channel is skipped.)

Math: the soft-histogram bin memberships telescope under cumsum:
    cumsum_k pj = sigmoid(2.5*t) - sigmoid(2.5*(t - (k+1))),  t = 255*x
so  cdf[k] = (T0 - F[k+1])/N  with  F[m] = sum_n sigmoid(2.5*(t_n - m)).

F is computed with a radix-16 decomposition: per-pixel block a = floor(t/16),
offset xi = t - 16a in [0,16).  22 sigmoid "slot" columns per pixel (scalar
engine) cover every non-saturated (pixel, bin) pair; a one-hot-over-blocks
matmul (tensor engine, contraction over pixels) bins them into per-block sums
G[16, 24]; saturated pairs are recovered exactly from block counts (suffix
sums).  Sharding: batch (8 samples) across 8 NeuronCores; each core handles
its sample's 4 channel-images.
"""
from contextlib import ExitStack

import numpy as np

import concourse.bass as bass
import concourse.tile as tile
from concourse import bacc, bass_utils, mybir

F32 = mybir.dt.float32
F16 = mybir.dt.float16
F8 = mybir.dt.float8e4
I32 = mybir.dt.int32

P = 128        # SBUF partitions
FD = 512       # free elems per partition per channel-image (128*512 = 65536)
NPIX = 65536
NCH = 4        # channel-images per core: [g, g_t, b, b_t]
N_CORES = 8

# Sigmoid slot columns: value sigmoid(2.5*(xi - off)).
#   cols 0..15:  off = b          (own block)
#   cols 16..18: off = 16 + b, b in {0,1,2}    (pixel one block below bin)
#   cols 19..21: off = b - 16, b in {13,14,15} (pixel one block above bin)
SLOT_OFFS = [float(b) for b in range(16)] + [16.0, 17.0, 18.0] + [-3.0, -2.0, -1.0]
COL_ONES = 22
NCOL = 23


def _colour_loss_kernel(
    ctx: ExitStack, tc: "tile.TileContext", out_ap, chs_ap, dbg_ap=None, reps=1
):
    nc = tc.nc
    AOP = mybir.AluOpType

    consts = ctx.enter_context(tc.tile_pool(name="consts", bufs=1))
    prep = ctx.enter_context(tc.tile_pool(name="prep", bufs=2))
    bigs = ctx.enter_context(tc.tile_pool(name="bigs", bufs=1))
    psums = ctx.enter_context(tc.tile_pool(name="psums", bufs=1, space="PSUM"))
    asm = ctx.enter_context(tc.tile_pool(name="asm", bufs=1))
    pools = (consts, prep, bigs, psums, asm)

    # per-slot activation biases (-2.5 * off), broadcast to all partitions
    nslot = len(SLOT_OFFS)
    bias_dram = nc.inline_tensor(
        np.array([[-2.5 * off for off in SLOT_OFFS]], dtype=np.float32),
        name="slotbias",
    ).ap()
    bias_sb = consts.tile([P, nslot], F32)
    nc.sync.dma_start(
        bias_sb[:],
        bass.AP(
            tensor=bias_dram.tensor,
            offset=bias_dram.offset,
            ap=[[0, P], bias_dram.ap[1]],
        ),
    )

    for _ in range(reps):
        _colour_loss_once(tc, out_ap, chs_ap, dbg_ap, pools, bias_sb)


def _colour_loss_once(tc, out_ap, chs_ap, dbg_ap, pools, bias_sb):
    nc = tc.nc
    AOP = mybir.AluOpType
    ACT = mybir.ActivationFunctionType
    consts, prep, bigs, psums, asm = pools
    FD2 = 2 * FD  # two channel-images per batch

    gpsums = [None] * NCH
    for q in range(2):  # pairs: (g, g_t), (b, b_t)
        xt = prep.tile([P, 2, FD], F32, tag="xt")
        nc.sync.dma_start(xt[:, 0, :], chs_ap[2 * q])
        nc.sync.dma_start(xt[:, 1, :], chs_ap[2 * q + 1])
        xtf = xt[:].rearrange("p h f -> p (h f)")
        t = prep.tile([P, FD2], F32, tag="t")
        nc.vector.tensor_scalar_mul(t[:], xtf, 255.0)
        tdiv = prep.tile([P, FD2], F32, tag="tdiv")
        nc.vector.tensor_scalar_mul(tdiv[:], t[:], 0.0625)
        a_i = prep.tile([P, FD2], I32, tag="a_i")
        nc.vector.tensor_copy(a_i[:], tdiv[:])  # trunc in sim, round-nearest on HW
        a_f0 = prep.tile([P, FD2], F32, tag="a_f0")
        nc.vector.tensor_copy(a_f0[:], a_i[:])
        xi0 = prep.tile([P, FD2], F32, tag="xi0")
        nc.vector.scalar_tensor_tensor(
            xi0[:], in0=a_f0[:], scalar=-16.0, in1=t[:],
            op0=AOP.mult, op1=AOP.add,
        )
        # fixup: convert may have given floor(t/16)+1 -> xi0 in [-16, 0).
        # neg = (xi0 < 0); a_f = a_f0 - neg; xi = xi0 + 16*neg
        neg = prep.tile([P, FD2], F32, tag="neg")
        nc.vector.tensor_scalar(neg[:], xi0[:], 0.0, None, op0=AOP.is_lt)
        a_f = prep.tile([P, FD2], F32, tag="a_f")
        nc.vector.tensor_tensor(a_f[:], a_f0[:], neg[:], op=AOP.subtract)
        xi = prep.tile([P, FD2], F32, tag="xi")
        nc.vector.scalar_tensor_tensor(
            xi[:], in0=neg[:], scalar=16.0, in1=xi0[:],
            op0=AOP.mult, op1=AOP.add,
        )

        # one-hot over blocks: ind[p, s, j] = (a[p, j] == s), f16 (exact 0/1).
        # fp8 out and gpsimd are both pathologically slow (7-16us/op measured);
        # f16-out is_equal on DVE runs at the fast packed path.
        ind = bigs.tile([P, 16, FD2], F16, tag="ind")
        for s in range(16):
            nc.vector.tensor_scalar(
                ind[:, s, :], a_f[:], float(s), None, op0=AOP.is_equal
            )

        # slot values
        u = bigs.tile([P, NCOL, FD2], F16, tag="u")
        for s in range(len(SLOT_OFFS)):
            nc.scalar.activation(
                u[:, s, :], xi[:], ACT.Sigmoid, bias=bias_sb[:, s : s + 1], scale=2.5
            )
        nc.vector.memset(u[:, COL_ONES, :], 1.0)

        # bin: G[a, col] = sum over pixels in block a of u[pixel, col]
        for h in range(2):
            i = 2 * q + h
            g = psums.tile([16, NCOL], F32, tag=f"g{i}")
            for j in range(FD):
                hj = h * FD + j
                nc.tensor.matmul(
                    g[:],
                    ind[:, :, hj : hj + 1],
                    u[:, :, hj : hj + 1],
                    start=(j == 0),
                    stop=(j == FD - 1),
                )
            gpsums[i] = g

    # ---- assembly ----
    gsb = []
    for i in range(NCH):
        s = asm.tile([16, NCOL], F32, tag=f"gsb{i}")
        nc.vector.tensor_copy(s[:], gpsums[i][:])
        if dbg_ap is not None:
            nc.sync.dma_start(dbg_ap[i], s[:])
        gsb.append(s)
    gd0 = asm.tile([16, NCOL], F32)
    nc.vector.tensor_tensor(gd0[:], gsb[0][:], gsb[1][:], op=AOP.subtract)
    gd1 = asm.tile([16, NCOL], F32)
    nc.vector.tensor_tensor(gd1[:], gsb[2][:], gsb[3][:], op=AOP.subtract)

    # flatten both pair-diffs onto partition 0: gdf[1, pair, A, col]
    gdf = asm.tile([1, 2, 16, NCOL], F32)
    nc.sync.dma_start(gdf[:, 0], gd0[:])
    nc.sync.dma_start(gdf[:, 1], gd1[:])

    # counts and suffix sums (zero-padded to 32)
    cnt0 = asm.tile([1, 2, 16], F32)
    nc.vector.tensor_copy(cnt0[:], gdf[:, :, :, COL_ONES])
    cntp = asm.tile([1, 2, 32], F32)
    nc.vector.memset(cntp[:], 0.0)
    nc.vector.tensor_copy(cntp[:, :, 0:16], cnt0[:])
    # in-place doubling: cntp[i] = sum_{j >= i} cnt[j]
    for k in (1, 2, 4, 8):
        nc.vector.tensor_tensor(
            cntp[:, :, 0 : 32 - k],
            cntp[:, :, 0 : 32 - k],
            cntp[:, :, k:32],
            op=AOP.add,
        )

    # D[m], m = 16A + b, laid out [1, pair, 256] with 4D view [1, pair, A, b]
    d = asm.tile([1, 2, 256], F32)
    d4 = d[:].rearrange("x p (A b) -> x p A b", b=16)
    nc.vector.tensor_copy(d4, gdf[:, :, :, 0:16])
    nc.vector.tensor_tensor(
        d4[:, :, 1:16, 0:3], d4[:, :, 1:16, 0:3], gdf[:, :, 0:15, 16:19], op=AOP.add
    )
    nc.vector.tensor_tensor(
        d4[:, :, 0:15, 13:16], d4[:, :, 0:15, 13:16], gdf[:, :, 1:16, 19:22], op=AOP.add
    )
    # + suffix count of blocks >= A+2
    nc.vector.tensor_tensor(
        d4, d4, cntp[:, :, 2:18].unsqueeze(3).broadcast_to([1, 2, 16, 16]), op=AOP.add
    )
    # + cnt[A+1] for b <= 12
    nc.vector.tensor_tensor(
        d4[:, :, 0:15, 0:13],
        d4[:, :, 0:15, 0:13],
        cnt0[:, :, 1:16].unsqueeze(3).broadcast_to([1, 2, 15, 13]),
        op=AOP.add,
    )

    # T0 diff per pair: T0 = (N - cnt[0]) + G[0, col0]  (sigmoid(2.5t) == 1.0
    # exactly in f32 for t >= 16, i.e. every pixel outside block 0)
    t0d = asm.tile([1, 2, 1], F32)
    nc.vector.tensor_tensor(
        t0d[:], gdf[:, :, 0, 0:1], cnt0[:, :, 0:1], op=AOP.subtract
    )
    invn = 1.0 / float(NPIX)
    t0dn = asm.tile([1, 2, 1], F32)
    nc.vector.tensor_scalar_mul(t0dn[:], t0d[:], invn)

    # delta[m] = (T0d - D[m])/N ; loss = sum over pairs, m=1..255 of delta^2
    tmp = asm.tile([1, 2, 256], F32)
    nc.vector.scalar_tensor_tensor(
        tmp[:], in0=d[:], scalar=-invn, in1=t0dn[:].broadcast_to([1, 2, 256]),
        op0=AOP.mult, op1=AOP.add,
    )
    dummy = asm.tile([1, 2, 255], F32)
    lossacc = asm.tile([1, 1], F32)
    nc.scalar.activation(
        dummy[:], tmp[:, :, 1:256], ACT.Square, accum_out=lossacc[:]
    )
    nc.sync.dma_start(out_ap[:], lossacc[:])


_CACHE: dict = {}


def build_nc(reps: int = 1):
    key = ("nc", reps)
    if key in _CACHE:
        return _CACHE[key]
    nc = bacc.Bacc(
        "TRN2", target_bir_lowering=False, debug=False, num_devices=N_CORES
    )
    chs = nc.dram_tensor("chs", [NCH, P, FD], F32, kind="ExternalInput").ap()
    out = nc.dram_tensor("out", [1, 1], F32, kind="ExternalOutput").ap()
    with tile.TileContext(nc) as tc:
        with ExitStack() as ctx:
            _colour_loss_kernel(ctx, tc, out, chs, reps=reps)
    nc.compile()
    _CACHE[key] = nc
    return nc


def make_in_maps(img: np.ndarray, img_t: np.ndarray):
    img = np.asarray(img)
    img_t = np.asarray(img_t)
    in_maps = []
    for c in range(N_CORES):
        chs = np.stack(
            [
                img[c, 1].reshape(P, FD),
                img_t[c, 1].reshape(P, FD),
                img[c, 2].reshape(P, FD),
                img_t[c, 2].reshape(P, FD),
            ],
            axis=0,
        ).astype(np.float32)
        in_maps.append({"chs": np.ascontiguousarray(chs)})
    return in_maps


def kernel(img: np.ndarray, img_t: np.ndarray, trace: bool = False):
    nc = build_nc()
    in_maps = make_in_maps(img, img_t)
    res = bass_utils.run_bass_kernel_spmd(
        nc, in_maps, core_ids=list(range(N_CORES)), trace=trace
    )
    out = np.array(
        [res.results[c]["out"][0, 0] for c in range(N_CORES)], dtype=np.float32
    )
    if trace:
        kernel.last_results = res  # type: ignore[attr-defined]
    return out



# revision 8
# speedup vs baseline: 2.1863x; 1.1554x over previous
"""ColourLoss Trainium2 kernel (self-contained).

Computes, per batch sample b:
    loss[b] = emd(hist_g(img), hist_g(img_t)) + emd(hist_b(img), hist_b(img_t))
(emd(r_hist, r_hist) == 0 exactly, so the r channel is skipped.)

Math: the soft-histogram bin memberships telescope under cumsum:
    cumsum_k pj = sigmoid(2.5*t) - sigmoid(2.5*(t - (k+1))),  t = 255*x
so  cdf[k] = (T0 - F[k+1])/N  with  F[m] = sum_n sigmoid(2.5*(t_n - m)).

F is computed with a radix-16 decomposition: per-pixel block a = floor(t/16),
offset xi = t - 16a in [0,16).  22 sigmoid "slot" columns per pixel (scalar
engine) cover every non-saturated (pixel, bin) pair; a one-hot-over-blocks
matmul (tensor engine, contraction over pixels) bins them into per-block sums
G[16, 24]; saturated pairs are recovered exactly from block counts (suffix
sums).  Sharding: batch (8 samples) across 8 NeuronCores; each core handles
its sample's 4 channel-images.
"""
from contextlib import ExitStack

import numpy as np

import concourse.bass as bass
import concourse.tile as tile
from concourse import bacc, bass_utils, mybir

F32 = mybir.dt.float32
F16 = mybir.dt.float16
F8 = mybir.dt.float8e4
I32 = mybir.dt.int32

P = 128        # SBUF partitions
FD = 512       # free elems per partition per channel-image (128*512 = 65536)
NPIX = 65536
NCH = 4        # channel-images per core: [g, g_t, b, b_t]
N_CORES = 8

# Sigmoid slot columns: value sigmoid(2.5*(xi - off)).
#   cols 0..15:  off = b          (own block)
#   cols 16..18: off = 16 + b, b in {0,1,2}    (pixel one block below bin)
#   cols 19..21: off = b - 16, b in {13,14,15} (pixel one block above bin)
SLOT_OFFS = [float(b) for b in range(16)] + [16.0, 17.0, 18.0] + [-3.0, -2.0, -1.0]
COL_ONES = 22
NCOL = 23


def _colour_loss_kernel(
    ctx: ExitStack, tc: "tile.TileContext", out_ap, chs_ap, dbg_ap=None, reps=1
):
    nc = tc.nc
    AOP = mybir.AluOpType

    consts = ctx.enter_context(tc.tile_pool(name="consts", bufs=1))
    prep = ctx.enter_context(tc.tile_pool(name="prep", bufs=2))
    bigs = ctx.enter_context(tc.tile_pool(name="bigs", bufs=2))
    psums = ctx.enter_context(tc.tile_pool(name="psums", bufs=1, space="PSUM"))
    asm = ctx.enter_context(tc.tile_pool(name="asm", bufs=1))
    pools = (consts, prep, bigs, psums, asm)

    # per-slot activation biases (-2.5 * off), broadcast to all partitions
    nslot = len(SLOT_OFFS)
    bias_dram = nc.inline_tensor(
        np.array([[-2.5 * off for off in SLOT_OFFS]], dtype=np.float32),
        name="slotbias",
    ).ap()
    bias_sb = consts.tile([P, nslot], F32)
    nc.sync.dma_start(
        bias_sb[:],
        bass.AP(
            tensor=bias_dram.tensor,
            offset=bias_dram.offset,
            ap=[[0, P], bias_dram.ap[1]],
        ),
    )

    for _ in range(reps):
        _colour_loss_once(tc, out_ap, chs_ap, dbg_ap, pools, bias_sb)


def _colour_loss_once(tc, out_ap, chs_ap, dbg_ap, pools, bias_sb):
    nc = tc.nc
    AOP = mybir.AluOpType
    ACT = mybir.ActivationFunctionType
    consts, prep, bigs, psums, asm = pools
    FD2 = 2 * FD  # two channel-images per batch

    gpsums = [None] * NCH
    for q in range(2):  # pairs: (g, g_t), (b, b_t)
        xt = prep.tile([P, 2, FD], F32, tag="xt")
        nc.sync.dma_start(xt[:, 0, :], chs_ap[2 * q])
        nc.sync.dma_start(xt[:, 1, :], chs_ap[2 * q + 1])
        xtf = xt[:].rearrange("p h f -> p (h f)")
        t = prep.tile([P, FD2], F32, tag="t")
        nc.vector.tensor_scalar_mul(t[:], xtf, 255.0)
        tdiv = prep.tile([P, FD2], F32, tag="tdiv")
        nc.vector.tensor_scalar_mul(tdiv[:], t[:], 0.0625)
        a_i = prep.tile([P, FD2], I32, tag="a_i")
        nc.vector.tensor_copy(a_i[:], tdiv[:])  # trunc in sim, round-nearest on HW
        a_f0 = prep.tile([P, FD2], F32, tag="a_f0")
        nc.vector.tensor_copy(a_f0[:], a_i[:])
        xi0 = prep.tile([P, FD2], F32, tag="xi0")
        nc.vector.scalar_tensor_tensor(
            xi0[:], in0=a_f0[:], scalar=-16.0, in1=t[:],
            op0=AOP.mult, op1=AOP.add,
        )
        # fixup: convert may have given floor(t/16)+1 -> xi0 in [-16, 0).
        # neg = (xi0 < 0); a_f = a_f0 - neg; xi = xi0 + 16*neg
        neg = prep.tile([P, FD2], F32, tag="neg")
        nc.vector.tensor_scalar(neg[:], xi0[:], 0.0, None, op0=AOP.is_lt)
        a_f = prep.tile([P, FD2], F32, tag="a_f")
        nc.vector.tensor_tensor(a_f[:], a_f0[:], neg[:], op=AOP.subtract)
        xi = prep.tile([P, FD2], F32, tag="xi")
        nc.vector.scalar_tensor_tensor(
            xi[:], in0=neg[:], scalar=16.0, in1=xi0[:],
            op0=AOP.mult, op1=AOP.add,
        )

        # slot values (per pair: 22 activations over FD2 amortize the 352-cyc
        # per-activation overhead)
        u = bigs.tile([P, NCOL, FD2], F16, tag="u")
        for s in range(len(SLOT_OFFS)):
            nc.scalar.activation(
                u[:, s, :], xi[:], ACT.Sigmoid, bias=bias_sb[:, s : s + 1], scale=2.5
            )
        nc.vector.memset(u[:, COL_ONES, :], 1.0)

        # one-hot over blocks: ind[p, s, j] = (a[p, j] == s), f16 (exact 0/1).
        # fp8 out and gpsimd are both pathologically slow (7-16us/op measured);
        # f16-out is_equal on DVE runs on the fast packed path.  Chunked per
        # channel-image (FD) so bufs=2 fits SBUF and the DVE builds chunk k+1
        # while the PE consumes chunk k.
        for h in range(2):
            i = 2 * q + h
            ind = bigs.tile([P, 16, FD], F16, tag="ind")
            for s in range(16):
                nc.vector.tensor_scalar(
                    ind[:, s, :], a_f[:, h * FD : (h + 1) * FD], float(s),
                    None, op0=AOP.is_equal
                )
            g = psums.tile([16, NCOL], F32, tag=f"g{i}")
            for j in range(FD):
                hj = h * FD + j
                nc.tensor.matmul(
                    g[:],
                    ind[:, :, j : j + 1],
                    u[:, :, hj : hj + 1],
                    start=(j == 0),
                    stop=(j == FD - 1),
                )
            gpsums[i] = g

    # ---- assembly ----
    gsb = []
    for i in range(NCH):
        s = asm.tile([16, NCOL], F32, tag=f"gsb{i}")
        nc.vector.tensor_copy(s[:], gpsums[i][:])
        if dbg_ap is not None:
            nc.sync.dma_start(dbg_ap[i], s[:])
        gsb.append(s)
    gd0 = asm.tile([16, NCOL], F32)
    nc.vector.tensor_tensor(gd0[:], gsb[0][:], gsb[1][:], op=AOP.subtract)
    gd1 = asm.tile([16, NCOL], F32)
    nc.vector.tensor_tensor(gd1[:], gsb[2][:], gsb[3][:], op=AOP.subtract)

    # flatten both pair-diffs onto partition 0: gdf[1, pair, A, col]
    gdf = asm.tile([1, 2, 16, NCOL], F32)
    nc.sync.dma_start(gdf[:, 0], gd0[:])
    nc.sync.dma_start(gdf[:, 1], gd1[:])

    # counts and suffix sums (zero-padded to 32)
    cnt0 = asm.tile([1, 2, 16], F32)
    nc.vector.tensor_copy(cnt0[:], gdf[:, :, :, COL_ONES])
    cntp = asm.tile([1, 2, 32], F32)
    nc.vector.memset(cntp[:], 0.0)
    nc.vector.tensor_copy(cntp[:, :, 0:16], cnt0[:])
    # in-place doubling: cntp[i] = sum_{j >= i} cnt[j]
    for k in (1, 2, 4, 8):
        nc.vector.tensor_tensor(
            cntp[:, :, 0 : 32 - k],
            cntp[:, :, 0 : 32 - k],
            cntp[:, :, k:32],
            op=AOP.add,
        )

    # D[m], m = 16A + b, laid out [1, pair, 256] with 4D view [1, pair, A, b]
    d = asm.tile([1, 2, 256], F32)
    d4 = d[:].rearrange("x p (A b) -> x p A b", b=16)
    nc.vector.tensor_copy(d4, gdf[:, :, :, 0:16])
    nc.vector.tensor_tensor(
        d4[:, :, 1:16, 0:3], d4[:, :, 1:16, 0:3], gdf[:, :, 0:15, 16:19], op=AOP.add
    )
    nc.vector.tensor_tensor(
        d4[:, :, 0:15, 13:16], d4[:, :, 0:15, 13:16], gdf[:, :, 1:16, 19:22], op=AOP.add
    )
    # + suffix count of blocks >= A+2
    nc.vector.tensor_tensor(
        d4, d4, cntp[:, :, 2:18].unsqueeze(3).broadcast_to([1, 2, 16, 16]), op=AOP.add
    )
    # + cnt[A+1] for b <= 12
    nc.vector.tensor_tensor(
        d4[:, :, 0:15, 0:13],
        d4[:, :, 0:15, 0:13],
        cnt0[:, :, 1:16].unsqueeze(3).broadcast_to([1, 2, 15, 13]),
        op=AOP.add,
    )

    # T0 diff per pair: T0 = (N - cnt[0]) + G[0, col0]  (sigmoid(2.5t) == 1.0
    # exactly in f32 for t >= 16, i.e. every pixel outside block 0)
    t0d = asm.tile([1, 2, 1], F32)
    nc.vector.tensor_tensor(
        t0d[:], gdf[:, :, 0, 0:1], cnt0[:, :, 0:1], op=AOP.subtract
    )
    invn = 1.0 / float(NPIX)
    t0dn = asm.tile([1, 2, 1], F32)
    nc.vector.tensor_scalar_mul(t0dn[:], t0d[:], invn)

    # delta[m] = (T0d - D[m])/N ; loss = sum over pairs, m=1..255 of delta^2
    tmp = asm.tile([1, 2, 256], F32)
    nc.vector.scalar_tensor_tensor(
        tmp[:], in0=d[:], scalar=-invn, in1=t0dn[:].broadcast_to([1, 2, 256]),
        op0=AOP.mult, op1=AOP.add,
    )
    dummy = asm.tile([1, 2, 255], F32)
    lossacc = asm.tile([1, 1], F32)
    nc.scalar.activation(
        dummy[:], tmp[:, :, 1:256], ACT.Square, accum_out=lossacc[:]
    )
    nc.sync.dma_start(out_ap[:], lossacc[:])


_CACHE: dict = {}


def build_nc(reps: int = 1):
    key = ("nc", reps)
    if key in _CACHE:
        return _CACHE[key]
    nc = bacc.Bacc(
        "TRN2", target_bir_lowering=False, debug=False, num_devices=N_CORES
    )
    chs = nc.dram_tensor("chs", [NCH, P, FD], F32, kind="ExternalInput").ap()
    out = nc.dram_tensor("out", [1, 1], F32, kind="ExternalOutput").ap()
    with tile.TileContext(nc) as tc:
        with ExitStack() as ctx:
            _colour_loss_kernel(ctx, tc, out, chs, reps=reps)
    nc.compile()
    _CACHE[key] = nc
    return nc


def make_in_maps(img: np.ndarray, img_t: np.ndarray):
    img = np.asarray(img)
    img_t = np.asarray(img_t)
    in_maps = []
    for c in range(N_CORES):
        chs = np.stack(
            [
                img[c, 1].reshape(P, FD),
                img_t[c, 1].reshape(P, FD),
                img[c, 2].reshape(P, FD),
                img_t[c, 2].reshape(P, FD),
            ],
            axis=0,
        ).astype(np.float32)
        in_maps.append({"chs": np.ascontiguousarray(chs)})
    return in_maps


def kernel(img: np.ndarray, img_t: np.ndarray, trace: bool = False):
    nc = build_nc()
    in_maps = make_in_maps(img, img_t)
    res = bass_utils.run_bass_kernel_spmd(
        nc, in_maps, core_ids=list(range(N_CORES)), trace=trace
    )
    out = np.array(
        [res.results[c]["out"][0, 0] for c in range(N_CORES)], dtype=np.float32
    )
    if trace:
        kernel.last_results = res  # type: ignore[attr-defined]
    return out



# revision 11
# speedup vs baseline: 2.4537x; 1.1223x over previous
"""ColourLoss Trainium2 kernel (self-contained).

Computes, per batch sample b:
    loss[b] = emd(hist_g(img), hist_g(img_t)) + emd(hist_b(img), hist_b(img_t))
(emd(r_hist, r_hist) == 0 exactly, so the r channel is skipped.)

Math: the soft-histogram bin memberships telescope under cumsum:
    cumsum_k pj = sigmoid(2.5*t) - sigmoid(2.5*(t - (k+1))),  t = 255*x
so  cdf[k] = (T0 - F[k+1])/N  with  F[m] = sum_n sigmoid(2.5*(t_n - m)).

F is computed with a radix-16 decomposition: per-pixel block a = floor(t/16),
offset xi = t - 16a in [0,16).  22 sigmoid "slot" columns per pixel (scalar
engine) cover every non-saturated (pixel, bin) pair; a one-hot-over-blocks
matmul (tensor engine, contraction over pixels) bins them into per-block sums
G[16, 24]; saturated pairs are recovered exactly from block counts (suffix
sums).  Sharding: batch (8 samples) across 8 NeuronCores; each core handles
its sample's 4 channel-images.
"""
from contextlib import ExitStack

import numpy as np

import concourse.bass as bass
import concourse.tile as tile
from concourse import bacc, bass_utils, mybir

F32 = mybir.dt.float32
F16 = mybir.dt.float16
F8 = mybir.dt.float8e4
I32 = mybir.dt.int32

P = 128        # SBUF partitions
FD = 512       # free elems per partition per channel-image (128*512 = 65536)
NPIX = 65536
NCH = 4        # channel-images per core: [g, g_t, b, b_t]
N_CORES = 8

# Sigmoid slot columns: value sigmoid(2.5*(xi - off)).
#   cols 0..15:  off = b          (own block)
#   cols 16..18: off = 16 + b, b in {0,1,2}    (pixel one block below bin)
#   cols 19..21: off = b - 16, b in {13,14,15} (pixel one block above bin)
SLOT_OFFS = [float(b) for b in range(16)] + [16.0, 17.0, 18.0] + [-3.0, -2.0, -1.0]
COL_ONES = 22
NCOL = 23


def _colour_loss_kernel(
    ctx: ExitStack, tc: "tile.TileContext", out_ap, chs_ap, dbg_ap=None, reps=1
):
    nc = tc.nc
    AOP = mybir.AluOpType

    consts = ctx.enter_context(tc.tile_pool(name="consts", bufs=1))
    prep = ctx.enter_context(tc.tile_pool(name="prep", bufs=2))
    bigs = ctx.enter_context(tc.tile_pool(name="bigs", bufs=2))
    psums = ctx.enter_context(tc.tile_pool(name="psums", bufs=1, space="PSUM"))
    asm = ctx.enter_context(tc.tile_pool(name="asm", bufs=1))
    pools = (consts, prep, bigs, psums, asm)

    # per-slot activation biases (-2.5 * off), broadcast to all partitions
    nslot = len(SLOT_OFFS)
    bias_dram = nc.inline_tensor(
        np.array([[-2.5 * off for off in SLOT_OFFS]], dtype=np.float32),
        name="slotbias",
    ).ap()
    bias_sb = consts.tile([P, nslot], F32)
    nc.sync.dma_start(
        bias_sb[:],
        bass.AP(
            tensor=bias_dram.tensor,
            offset=bias_dram.offset,
            ap=[[0, P], bias_dram.ap[1]],
        ),
    )

    for _ in range(reps):
        _colour_loss_once(tc, out_ap, chs_ap, dbg_ap, pools, bias_sb)


def _colour_loss_once(tc, out_ap, chs_ap, dbg_ap, pools, bias_sb):
    nc = tc.nc
    AOP = mybir.AluOpType
    ACT = mybir.ActivationFunctionType
    consts, prep, bigs, psums, asm = pools
    FD2 = 2 * FD  # two channel-images per batch

    gpsums = [None] * NCH
    for q in range(2):  # pairs: (g, g_t), (b, b_t)
        xt = prep.tile([P, 2, FD], F32, tag="xt")
        nc.sync.dma_start(xt[:, 0, :], chs_ap[2 * q])
        nc.sync.dma_start(xt[:, 1, :], chs_ap[2 * q + 1])
        xtf = xt[:].rearrange("p h f -> p (h f)")
        t = prep.tile([P, FD2], F32, tag="t")
        nc.vector.tensor_scalar_mul(t[:], xtf, 255.0)
        tdiv = prep.tile([P, FD2], F32, tag="tdiv")
        nc.vector.tensor_scalar_mul(tdiv[:], t[:], 0.0625)
        a_i = prep.tile([P, FD2], I32, tag="a_i")
        nc.vector.tensor_copy(a_i[:], tdiv[:])  # trunc in sim, round-nearest on HW
        a_f0 = prep.tile([P, FD2], F32, tag="a_f0")
        nc.vector.tensor_copy(a_f0[:], a_i[:])
        xi0 = prep.tile([P, FD2], F32, tag="xi0")
        nc.vector.scalar_tensor_tensor(
            xi0[:], in0=a_f0[:], scalar=-16.0, in1=t[:],
            op0=AOP.mult, op1=AOP.add,
        )
        # fixup: convert may have given floor(t/16)+1 -> xi0 in [-16, 0).
        # neg = (xi0 < 0); a_f = a_f0 - neg; xi = xi0 + 16*neg
        neg = prep.tile([P, FD2], F32, tag="neg")
        nc.vector.tensor_scalar(neg[:], xi0[:], 0.0, None, op0=AOP.is_lt)
        a_f = prep.tile([P, FD2], F32, tag="a_f")
        nc.vector.tensor_tensor(a_f[:], a_f0[:], neg[:], op=AOP.subtract)
        xi = prep.tile([P, FD2], F32, tag="xi")
        nc.vector.scalar_tensor_tensor(
            xi[:], in0=neg[:], scalar=16.0, in1=xi0[:],
            op0=AOP.mult, op1=AOP.add,
        )

        # slot values (per pair: 22 activations over FD2 amortize the 352-cyc
        # per-activation overhead)
        u = bigs.tile([P, NCOL, FD2], F16, tag="u")
        for s in range(len(SLOT_OFFS)):
            nc.scalar.activation(
                u[:, s, :], xi[:], ACT.Sigmoid, bias=bias_sb[:, s : s + 1], scale=2.5
            )
        nc.vector.memset(u[:, COL_ONES, :], 1.0)

        # one-hot over blocks: ind[p, s, j] = (a[p, j] == s), f16 (exact 0/1).
        # fp8 out and gpsimd are both pathologically slow (7-16us/op measured);
        # f16-out is_equal on DVE runs on the fast packed path.  Chunked per
        # channel-image (FD) so bufs=2 fits SBUF and the DVE builds chunk k+1
        # while the PE consumes chunk k.
        for h in range(2):
            i = 2 * q + h
            ind = bigs.tile([P, 16, FD], F16, tag="ind")
            for s in range(16):
                nc.vector.tensor_scalar(
                    ind[:, s, :], a_f[:, h * FD : (h + 1) * FD], float(s),
                    None, op0=AOP.is_equal
                )
            # round-robin the accumulation over 4 PE column strips
            # (tile_position) so up to 4 LDW+MM pairs run concurrently.
            g = psums.tile([P, NCOL], F32, tag=f"g{i}")
            for j in range(FD):
                strip = j & 3
                hj = h * FD + j
                nc.tensor.matmul(
                    g[32 * strip : 32 * strip + 16, :],
                    ind[:, :, j : j + 1],
                    u[:, :, hj : hj + 1],
                    start=(j < 4),
                    stop=(j >= FD - 4),
                    tile_position=(0, 32 * strip),
                )
            gpsums[i] = g

    # ---- assembly ----
    gsb = []
    for i in range(NCH):
        s = asm.tile([16, NCOL], F32, tag=f"gsb{i}")
        nc.vector.tensor_copy(s[:], gpsums[i][0:16, :])
        for st in range(1, 4):
            nc.vector.tensor_tensor(
                s[:], s[:], gpsums[i][32 * st : 32 * st + 16, :], op=AOP.add
            )
        if dbg_ap is not None:
            nc.sync.dma_start(dbg_ap[i], s[:])
        gsb.append(s)
    gd0 = asm.tile([16, NCOL], F32)
    nc.vector.tensor_tensor(gd0[:], gsb[0][:], gsb[1][:], op=AOP.subtract)
    gd1 = asm.tile([16, NCOL], F32)
    nc.vector.tensor_tensor(gd1[:], gsb[2][:], gsb[3][:], op=AOP.subtract)

    # flatten both pair-diffs onto partition 0: gdf[1, pair, A, col]
    gdf = asm.tile([1, 2, 16, NCOL], F32)
    nc.sync.dma_start(gdf[:, 0], gd0[:])
    nc.sync.dma_start(gdf[:, 1], gd1[:])

    # counts and suffix sums (zero-padded to 32)
    cnt0 = asm.tile([1, 2, 16], F32)
    nc.vector.tensor_copy(cnt0[:], gdf[:, :, :, COL_ONES])
    cntp = asm.tile([1, 2, 32], F32)
    nc.vector.memset(cntp[:], 0.0)
    nc.vector.tensor_copy(cntp[:, :, 0:16], cnt0[:])
    # in-place doubling: cntp[i] = sum_{j >= i} cnt[j]
    for k in (1, 2, 4, 8):
        nc.vector.tensor_tensor(
            cntp[:, :, 0 : 32 - k],
            cntp[:, :, 0 : 32 - k],
            cntp[:, :, k:32],
            op=AOP.add,
        )

    # D[m], m = 16A + b, laid out [1, pair, 256] with 4D view [1, pair, A, b]
    d = asm.tile([1, 2, 256], F32)
    d4 = d[:].rearrange("x p (A b) -> x p A b", b=16)
    nc.vector.tensor_copy(d4, gdf[:, :, :, 0:16])
    nc.vector.tensor_tensor(
        d4[:, :, 1:16, 0:3], d4[:, :, 1:16, 0:3], gdf[:, :, 0:15, 16:19], op=AOP.add
    )
    nc.vector.tensor_tensor(
        d4[:, :, 0:15, 13:16], d4[:, :, 0:15, 13:16], gdf[:, :, 1:16, 19:22], op=AOP.add
    )
    # + suffix count of blocks >= A+2
    nc.vector.tensor_tensor(
        d4, d4, cntp[:, :, 2:18].unsqueeze(3).broadcast_to([1, 2, 16, 16]), op=AOP.add
    )
    # + cnt[A+1] for b <= 12
    nc.vector.tensor_tensor(
        d4[:, :, 0:15, 0:13],
        d4[:, :, 0:15, 0:13],
        cnt0[:, :, 1:16].unsqueeze(3).broadcast_to([1, 2, 15, 13]),
        op=AOP.add,
    )

    # T0 diff per pair: T0 = (N - cnt[0]) + G[0, col0]  (sigmoid(2.5t) == 1.0
    # exactly in f32 for t >= 16, i.e. every pixel outside block 0)
    t0d = asm.tile([1, 2, 1], F32)
    nc.vector.tensor_tensor(
        t0d[:], gdf[:, :, 0, 0:1], cnt0[:, :, 0:1], op=AOP.subtract
    )
    invn = 1.0 / float(NPIX)
    t0dn = asm.tile([1, 2, 1], F32)
    nc.vector.tensor_scalar_mul(t0dn[:], t0d[:], invn)

    # delta[m] = (T0d - D[m])/N ; loss = sum over pairs, m=1..255 of delta^2
    tmp = asm.tile([1, 2, 256], F32)
    nc.vector.scalar_tensor_tensor(
        tmp[:], in0=d[:], scalar=-invn, in1=t0dn[:].broadcast_to([1, 2, 256]),
        op0=AOP.mult, op1=AOP.add,
    )
    dummy = asm.tile([1, 2, 255], F32)
    lossacc = asm.tile([1, 1], F32)
    nc.scalar.activation(
        dummy[:], tmp[:, :, 1:256], ACT.Square, accum_out=lossacc[:]
    )
    nc.sync.dma_start(out_ap[:], lossacc[:])


_CACHE: dict = {}


def build_nc(reps: int = 1):
    key = ("nc", reps)
    if key in _CACHE:
        return _CACHE[key]
    nc = bacc.Bacc(
        "TRN2", target_bir_lowering=False, debug=False, num_devices=N_CORES
    )
    chs = nc.dram_tensor("chs", [NCH, P, FD], F32, kind="ExternalInput").ap()
    out = nc.dram_tensor("out", [1, 1], F32, kind="ExternalOutput").ap()
    with tile.TileContext(nc) as tc:
        with ExitStack() as ctx:
            _colour_loss_kernel(ctx, tc, out, chs, reps=reps)
    nc.compile()
    _CACHE[key] = nc
    return nc


def make_in_maps(img: np.ndarray, img_t: np.ndarray):
    img = np.asarray(img)
    img_t = np.asarray(img_t)
    in_maps = []
    for c in range(N_CORES):
        chs = np.stack(
            [
                img[c, 1].reshape(P, FD),
                img_t[c, 1].reshape(P, FD),
                img[c, 2].reshape(P, FD),
                img_t[c, 2].reshape(P, FD),
            ],
            axis=0,
        ).astype(np.float32)
        in_maps.append({"chs": np.ascontiguousarray(chs)})
    return in_maps


def kernel(img: np.ndarray, img_t: np.ndarray, trace: bool = False):
    nc = build_nc()
    in_maps = make_in_maps(img, img_t)
    res = bass_utils.run_bass_kernel_spmd(
        nc, in_maps, core_ids=list(range(N_CORES)), trace=trace
    )
    out = np.array(
        [res.results[c]["out"][0, 0] for c in range(N_CORES)], dtype=np.float32
    )
    if trace:
        kernel.last_results = res  # type: ignore[attr-defined]
    return out

